# revision 6
# baseline (speedup 1.0000x reference)
"""Trainium2 Bass kernel for nn_CombinedLoss (debiased Sinkhorn divergence +
energy distance) on 8 NeuronCores.

Strategy (scheme A): every core owns a 512-row slice of all four Sinkhorn
potentials (f, g, sx, sy). Each of the 30 eps-annealing steps runs four
softmin row-blocks [512, 4096] per core; the [N,M] kernels are recomputed on
the fly as bf16 split-K=98 augmented matmuls P = S + v (S = Gram, v = pot - zn
folded in via two extra contraction rows). The per-row logsumexp shift is the
previous step's row-lse (validated margin ~42 << 88), which removes the
max-reduce pass entirely: ACT does exp+accumulate straight out of PSUM.
Potentials are exchanged with two 8-core AllGathers per step, pipelined behind
the other block pair's compute. Energy distance runs as a final fp32-matmul +
sqrt-accumulate phase. Final means are combined with one AllReduce.
"""
import numpy as np
import ml_dtypes
import concourse.bacc as bacc
import concourse.mybir as mybir
import concourse.tile as tile
from concourse.bass_utils import run_bass_kernel_spmd

F32 = mybir.dt.float32
BF16 = mybir.dt.bfloat16
AX = mybir.AxisListType
OP = mybir.AluOpType
AF = mybir.ActivationFunctionType

N = 4096
D = 32
NCORES = 8
ROWS = N // NCORES          # 512
NT = ROWS // 128            # 4 row tiles per core
NSTEP = 30
EPS_FIN = float(np.float32(0.05) ** 2)      # blur**2
RATIO = float(np.float32(0.5) ** 2)         # scaling**2
LOGN = float(np.log(np.float64(N)).astype(np.float32))

_CACHE = {}


def _patched_tables(orig, arch):
    """Keep every table set (ids must stay aligned with act_info.json), but
    empty out all sets except the exp+ln combined set and the sqrt set, so the
    table chooser can only pick those and never thrashes reloads."""
    full = orig(arch)
    keep = ("natural_log_exp_and_others", "sqrt_and_others")
    if not all(k in full for k in keep):
        return full
    return {name: (s if name in keep else set()) for name, s in full.items()}


def _build_nc():
    import concourse.bacc as _bacc_mod
    orig = _bacc_mod.get_activation_tables
    _bacc_mod.get_activation_tables = lambda arch: _patched_tables(orig, arch)
    try:
        return _build_nc_inner()
    finally:
        _bacc_mod.get_activation_tables = orig


def _build_nc_inner():
    nc = bacc.Bacc("TRN2", target_bir_lowering=False, num_devices=NCORES)

    # ---- per-core inputs ----
    lin_x = nc.declare_dram_parameter("lin_x", [98, 512], BF16, isOutput=False)
    lin_y = nc.declare_dram_parameter("lin_y", [98, 512], BF16, isOutput=False)
    rzx = nc.declare_dram_parameter("rzx", [96, 4096], BF16, isOutput=False)
    rzy = nc.declare_dram_parameter("rzy", [96, 4096], BF16, isOutput=False)
    znx = nc.declare_dram_parameter("znx", [128, 32], F32, isOutput=False)
    zny = nc.declare_dram_parameter("zny", [128, 32], F32, isOutput=False)
    rnx = nc.declare_dram_parameter("rnx", [128, 4], F32, isOutput=False)
    rny = nc.declare_dram_parameter("rny", [128, 4], F32, isOutput=False)
    elin_x = nc.declare_dram_parameter("elin_x", [33, 512], F32, isOutput=False)
    elin_y = nc.declare_dram_parameter("elin_y", [33, 512], F32, isOutput=False)
    erx = nc.declare_dram_parameter("erx", [33, 4096], F32, isOutput=False)
    ery = nc.declare_dram_parameter("ery", [33, 4096], F32, isOutput=False)
    rpow = nc.declare_dram_parameter("rpow", [1, 32], F32, isOutput=False)
    wvec = nc.declare_dram_parameter("wvec", [1, 8], F32, isOutput=False)
    out = nc.declare_dram_parameter("out", [1, 1], F32, isOutput=True)

    with tile.TileContext(nc) as tc:
        with (
            tc.tile_pool(name="sb", bufs=1) as sb,
            tc.tile_pool(name="sc", bufs=2) as sc,          # cycling scratch
            tc.tile_pool(name="dr", bufs=2, space="DRAM") as dr,
        ):
            # ---- persistent SBUF tiles ----
            linx = sb.tile([98, 512], BF16, tag="linx")
            liny = sb.tile([98, 512], BF16, tag="liny")
            rf = sb.tile([98, 4096], BF16, tag="rf")     # Z=y, pot g
            rg = sb.tile([98, 4096], BF16, tag="rg")     # Z=x, pot f
            rsx = sb.tile([98, 4096], BF16, tag="rsx")   # Z=x, pot sx
            rsy = sb.tile([98, 4096], BF16, tag="rsy")   # Z=y, pot sy
            znxT = sb.tile([128, 32], F32, tag="znxT")
            znyT = sb.tile([128, 32], F32, tag="znyT")
            rnxT = sb.tile([128, 4], F32, tag="rnxT")
            rnyT = sb.tile([128, 4], F32, tag="rnyT")
            elinx = sb.tile([33, 512], F32, tag="elinx")
            eliny = sb.tile([33, 512], F32, tag="eliny")
            erxT = sb.tile([33, 4096], F32, tag="erxT")
            eryT = sb.tile([33, 4096], F32, tag="eryT")
            potA = sb.tile([128, 64], F32, tag="potA")   # cols 0-31 f, 32-63 g
            potB = sb.tile([128, 64], F32, tag="potB")   # cols 0-31 sx, 32-63 sy
            pmine = sb.tile([128, 16], F32, tag="pmine")  # f|g|sx|sy 4 cols each
            lsep = sb.tile([128, 16], F32, tag="lsep")    # prev lse (shift source)
            rfin = sb.tile([128, 16], F32, tag="rfin")
            SA = sb.tile([128, 16], F32, tag="SA")        # half-0 exp sums
            SB = sb.tile([128, 16], F32, tag="SB")        # half-1 exp sums
            biasT = sb.tile([128, 16], F32, tag="biasT")
            esum = sb.tile([128, 24], F32, tag="esum")    # energy accums
            erow = sb.tile([1, 32], F32, tag="erow")
            invrow = sb.tile([1, 32], F32, tag="invrow")
            epsb = sb.tile([128, 32], F32, tag="epsb")
            invb = sb.tile([128, 32], F32, tag="invb")
            ninvb = sb.tile([128, 32], F32, tag="ninvb")
            rprow = sb.tile([1, 32], F32, tag="rprow")
            wrow = sb.tile([1, 8], F32, tag="wrow")
            ones1 = sb.tile([128, 1], F32, tag="ones1")
            rn2x = sb.tile([128, 4], F32, tag="rn2x")
            rn2y = sb.tile([128, 4], F32, tag="rn2y")
            dmin = sb.tile([128, 8], F32, tag="dmin")
            diam_sb = sb.tile([1, 1], F32, tag="diam_sb")

            # ---- loads ----
            nc.sync.dma_start(linx[:], lin_x[:])
            nc.sync.dma_start(liny[:], lin_y[:])
            nc.sync.dma_start(rf[0:96, :], rzy[:])
            nc.sync.dma_start(rg[0:96, :], rzx[:])
            nc.sync.dma_start(rsx[0:96, :], rzx[:])
            nc.sync.dma_start(rsy[0:96, :], rzy[:])
            nc.sync.dma_start(erxT[:], erx[:])
            nc.sync.dma_start(eryT[:], ery[:])
            nc.sync.dma_start(elinx[:], elin_x[:])
            nc.sync.dma_start(eliny[:], elin_y[:])
            nc.gpsimd.dma_start(znxT[:], znx[:])
            nc.gpsimd.dma_start(znyT[:], zny[:])
            nc.gpsimd.dma_start(rnxT[:], rnx[:])
            nc.gpsimd.dma_start(rnyT[:], rny[:])
            nc.gpsimd.dma_start(rprow[:], rpow[:])
            nc.gpsimd.dma_start(wrow[:], wvec[:])
            nc.vector.memset(potA[:], 0.0)
            nc.vector.memset(potB[:], 0.0)
            nc.vector.memset(pmine[:], 0.0)
            nc.vector.memset(lsep[:], 0.0)
            nc.vector.memset(ones1[:], 1.0)
            nc.vector.tensor_scalar_mul(rn2x[:], rnxT[:], 2.0)
            nc.vector.tensor_scalar_mul(rn2y[:], rnyT[:], 2.0)

            # block descriptors: (rhs, lin, zn, rn, pot-col-slice, col-base)
            BLK = [
                ("f", rf, linx, znyT, rnxT, potA, 32, 0),
                ("g", rg, liny, znxT, rnyT, potA, 0, 4),
                ("sx", rsx, linx, znxT, rnxT, potB, 0, 8),
                ("sy", rsy, liny, znyT, rnyT, potB, 32, 12),
            ]

            def v_prep(b):
                _, rhs, _, zn, _, pot, pc, _ = BLK[b]
                v = sc.tile([128, 32], F32, tag="v")
                nc.vector.tensor_tensor(v[:], pot[:, pc:pc + 32], zn[:],
                                        op=OP.subtract)
                vst = sc.tile([128, 64], BF16, tag="vst")
                nc.vector.tensor_copy(vst[:, 0:32], v[:])
                nc.vector.tensor_tensor(vst[:, 32:64], v[:], vst[:, 0:32],
                                        op=OP.subtract)
                vb = dr.tile([128, 64], BF16, tag="vb")
                nc.sync.dma_start(vb[:], vst[:])
                nc.sync.dma_start(
                    rhs[96:98, :].rearrange("h (p e) -> h p e", p=128),
                    vb[:].rearrange("p (h e) -> h p e", h=2),
                )

            def softmin_block(b, k, psA, fin=False):
                """One softmin row-block at eps slot k. Results into pmine
                (scan update) or rfin (final extrapolation)."""
                name, rhs, lin, zn, rn, pot, pc, cb = BLK[b]
                v_prep(b)
                # bias = -lse_prev * inv_eps
                nc.vector.tensor_scalar(
                    biasT[:, cb:cb + 4], lsep[:, cb:cb + 4], ninvb[:, k:k + 1],
                    None, op0=OP.mult)
                for t in range(NT):
                    for h in range(2):
                        mm = psA.tile([128, 2048], F32, tag="mm")
                        for j in range(4):
                            nc.tensor.matmul(
                                mm[:, 512 * j:512 * (j + 1)],
                                lin[:, 128 * t:128 * (t + 1)],
                                rhs[:, 2048 * h + 512 * j:2048 * h + 512 * (j + 1)],
                                start=True, stop=True)
                        esc = sc.tile([128, 2048], BF16, tag="esc")
                        acc = (SA if h == 0 else SB)
                        nc.scalar.activation(
                            esc[:], mm[:], AF.Exp,
                            bias=biasT[:, cb + t:cb + t + 1],
                            scale=invb[:, k:k + 1],
                            accum_out=acc[:, cb + t:cb + t + 1])
                # finalize: lse_new = lse_prev + eps*(ln(sA+sB) - logN)
                s4 = sc.tile([128, 4], F32, tag="s4")
                nc.vector.tensor_tensor(s4[:], SA[:, cb:cb + 4], SB[:, cb:cb + 4],
                                        op=OP.add)
                l4 = sc.tile([128, 4], F32, tag="l4")
                nc.scalar.activation(l4[:], s4[:], AF.Ln)
                t4 = sc.tile([128, 4], F32, tag="t4")
                nc.vector.tensor_scalar_sub(t4[:], l4[:], LOGN)
                nc.vector.tensor_scalar(t4[:], t4[:], epsb[:, k:k + 1], None,
                                        op0=OP.mult)
                nc.vector.tensor_tensor(lsep[:, cb:cb + 4], t4[:],
                                        lsep[:, cb:cb + 4], op=OP.add)
                dst = rfin if fin else None
                if fin:
                    nc.vector.tensor_tensor(rfin[:, cb:cb + 4], rn[:],
                                            lsep[:, cb:cb + 4], op=OP.subtract)
                else:
                    r4 = sc.tile([128, 4], F32, tag="r4")
                    nc.vector.tensor_tensor(r4[:], rn[:], lsep[:, cb:cb + 4],
                                            op=OP.subtract)
                    nc.vector.tensor_tensor(pmine[:, cb:cb + 4],
                                            pmine[:, cb:cb + 4], r4[:], op=OP.add)
                    nc.vector.tensor_scalar_mul(pmine[:, cb:cb + 4],
                                                pmine[:, cb:cb + 4], 0.5)

            def gather(grp):
                """AllGather the (f,g) or (sx,sy) slice pair and refresh pot."""
                lo = 0 if grp == 0 else 8
                pot = potA if grp == 0 else potB
                cin = dr.tile([2, 512], F32, tag=f"cin{grp}")
                nc.sync.dma_start(
                    cin[:].rearrange("b (p e) -> p b e", p=128),
                    pmine[:, lo:lo + 8].rearrange("p (b e) -> p b e", b=2))
                gout = dr.tile([8, 2, 512], F32, tag=f"gout{grp}")
                nc.gpsimd.collective_compute(
                    "AllGather", OP.bypass,
                    replica_groups=[list(range(NCORES))],
                    ins=[cin.opt()], outs=[gout.opt()])
                plin = dr.tile([2, 4096], F32, tag=f"plin{grp}")
                nc.sync.dma_start(
                    plin[:].rearrange("b (r q) -> r b q", r=8), gout[:])
                nc.sync.dma_start(
                    pot[:].rearrange("p (b e) -> p b e", b=2),
                    plin[:].rearrange("b (p e) -> p b e", p=128))

            with tc.tile_pool(name="psA", bufs=2, space="PSUM") as psA:
                # ---- diam pass: min_j P over the f-block (pot=0 -> v=-zn_y) ----
                v_prep(0)
                for t in range(NT):
                    for h in range(2):
                        mm = psA.tile([128, 2048], F32, tag="mm")
                        for j in range(4):
                            nc.tensor.matmul(
                                mm[:, 512 * j:512 * (j + 1)],
                                linx[:, 128 * t:128 * (t + 1)],
                                rf[:, 2048 * h + 512 * j:2048 * h + 512 * (j + 1)],
                                start=True, stop=True)
                        nc.vector.tensor_reduce(dmin[:, 2 * t + h:2 * t + h + 1],
                                                mm[:], axis=AX.X, op=OP.min)
                dmin4 = sc.tile([128, 4], F32, tag="dmin4")
                nc.vector.tensor_reduce(
                    dmin4[:], dmin[:].rearrange("p (t h) -> p t h", h=2),
                    axis=AX.X, op=OP.min)
                cand = sc.tile([128, 4], F32, tag="cand")
                nc.vector.tensor_tensor(cand[:], rnxT[:], dmin4[:], op=OP.subtract)
                dc1 = sc.tile([128, 1], F32, tag="dc1")
                nc.vector.tensor_reduce(dc1[:], cand[:], axis=AX.X, op=OP.max)
                dstage = dr.tile([1, 128], F32, tag="dstage")
                nc.sync.dma_start(
                    dstage[:].rearrange("o (p e) -> (o p) e", p=128), dc1[:])
                dr128 = sc.tile([1, 128], F32, tag="dr128")
                nc.sync.dma_start(dr128[:], dstage[:])
                dmax = sc.tile([1, 1], F32, tag="dmax")
                nc.vector.tensor_reduce(dmax[:], dr128[:], axis=AX.X, op=OP.max)
                dIn = dr.tile([1, 1], F32, tag="dIn")
                dOut = dr.tile([1, 1], F32, tag="dOut")
                nc.sync.dma_start(dIn[:], dmax[:])
                nc.gpsimd.collective_compute(
                    "AllReduce", OP.max,
                    replica_groups=[list(range(NCORES))],
                    ins=[dIn.opt()], outs=[dOut.opt()])
                nc.sync.dma_start(diam_sb[:], dOut[:])

                # ---- eps schedule ----
                nc.vector.tensor_scalar(erow[:], rprow[:], diam_sb[:], EPS_FIN,
                                        op0=OP.mult, op1=OP.max)
                nc.vector.reciprocal(invrow[:], erow[:])
                nc.gpsimd.partition_broadcast(epsb[:], erow[:])
                nc.gpsimd.partition_broadcast(invb[:], invrow[:])
                nc.vector.tensor_scalar_mul(ninvb[:], invb[:], -1.0)

                # ---- 30 sinkhorn steps ----
                for k in range(NSTEP):
                    softmin_block(0, k, psA)
                    softmin_block(1, k, psA)
                    gather(0)
                    softmin_block(2, k, psA)
                    softmin_block(3, k, psA)
                    gather(1)

                # ---- final extrapolation at eps_fin (slot 30) ----
                for b in range(4):
                    softmin_block(b, 30, psA, fin=True)

                # ---- energy distance ----
                EMAT = [(elinx, eryT, rn2x, False),   # d(x,y)
                        (elinx, erxT, rn2x, True),    # d(x,x)
                        (eliny, eryT, rn2y, True)]    # d(y,y)
                for m, (elin, er, rn2, clamp) in enumerate(EMAT):
                    for t in range(NT):
                        for h in range(2):
                            mm = psA.tile([128, 2048], F32, tag="mm")
                            for j in range(4):
                                nc.tensor.matmul(
                                    mm[:, 512 * j:512 * (j + 1)],
                                    elin[:, 128 * t:128 * (t + 1)],
                                    er[:, 2048 * h + 512 * j:2048 * h + 512 * (j + 1)],
                                    start=True, stop=True)
                            col = 8 * m + 2 * t + h
                            if clamp:
                                nc.vector.tensor_scalar(
                                    mm[:], mm[:], rn2[:, t:t + 1], 1e-12,
                                    op0=OP.add, op1=OP.max)
                                escf = sc.tile([128, 2048], F32, tag="escf")
                                nc.scalar.activation(
                                    escf[:], mm[:], AF.Sqrt,
                                    accum_out=esum[:, col:col + 1])
                            else:
                                escf = sc.tile([128, 2048], F32, tag="escf")
                                nc.scalar.activation(
                                    escf[:], mm[:], AF.Sqrt,
                                    bias=rn2[:, t:t + 1],
                                    accum_out=esum[:, col:col + 1])

            # ---- final reductions ----
            pack = sb.tile([128, 40], F32, tag="pack")
            nc.vector.tensor_copy(pack[:, 0:16], rfin[:])
            nc.vector.tensor_copy(pack[:, 16:40], esum[:])
            with tc.tile_pool(name="psB", bufs=1, space="PSUM") as psB:
                pout = psB.tile([1, 40], F32, tag="pout")
                nc.tensor.matmul(pout[:], ones1[:], pack[:], start=True, stop=True)
                row40 = sc.tile([1, 40], F32, tag="row40")
                nc.scalar.copy(row40[:], pout[:])
            r10 = sc.tile([1, 10], F32, tag="r10")
            nc.vector.tensor_reduce(
                r10[:], row40[:].rearrange("o (g q) -> o g q", q=4),
                axis=AX.X, op=OP.add)
            part8 = sc.tile([1, 8], F32, tag="part8")
            nc.vector.memset(part8[:], 0.0)
            nc.vector.tensor_copy(part8[:, 0:4], r10[:, 0:4])
            nc.vector.tensor_reduce(
                part8[:, 4:7], r10[:, 4:10].rearrange("o (g q) -> o g q", q=2),
                axis=AX.X, op=OP.add)
            pIn = dr.tile([1, 8], F32, tag="pIn")
            pOut = dr.tile([1, 8], F32, tag="pOut")
            nc.sync.dma_start(pIn[:], part8[:])
            nc.gpsimd.collective_compute(
                "AllReduce", OP.add,
                replica_groups=[list(range(NCORES))],
                ins=[pIn.opt()], outs=[pOut.opt()])
            row8 = sc.tile([1, 8], F32, tag="row8")
            nc.sync.dma_start(row8[:], pOut[:])
            nc.vector.tensor_tensor(row8[:], row8[:], wrow[:], op=OP.mult)
            res = sc.tile([1, 1], F32, tag="res")
            nc.vector.tensor_reduce(res[:], row8[:], axis=AX.X, op=OP.add)
            nc.sync.dma_start(out[:], res[:])

    nc.finalize()
    return nc


def _bf16(a):
    return np.asarray(a, np.float32).astype(ml_dtypes.bfloat16)


def _split2(a):
    h = _bf16(a)
    l = _bf16(np.asarray(a, np.float32) - h.astype(np.float32))
    return h, l


def _build_inputs(x, y):
    # permutation rho: gathered index j = 512c + 4p + t <-> row 512c + 128t + p
    c = np.arange(N) // ROWS
    r = np.arange(N) % ROWS
    p, t = r // 4, r % 4
    rho = ROWS * c + 128 * t + p
    xp, yp = x[rho], y[rho]

    znx_v = (0.5 * np.sum(xp * xp, 1)).astype(np.float32)
    zny_v = (0.5 * np.sum(yp * yp, 1)).astype(np.float32)
    ones2 = np.ones((2, 512), np.float32)

    xph, xpl = _split2(xp)
    yph, ypl = _split2(yp)
    rzx = np.vstack([xph.T, xph.T, xpl.T]).astype(ml_dtypes.bfloat16)
    rzy = np.vstack([yph.T, yph.T, ypl.T]).astype(ml_dtypes.bfloat16)
    erx = np.vstack([xp.T.astype(np.float32), (2 * znx_v)[None, :]]).astype(np.float32)
    ery = np.vstack([yp.T.astype(np.float32), (2 * zny_v)[None, :]]).astype(np.float32)

    rpow = np.zeros((1, 32), np.float32)
    rpow[0, :NSTEP] = np.float32(RATIO) ** np.arange(NSTEP, dtype=np.float32)
    s = np.float32(0.001) / N
    u = np.float32(1.0) / (N * N)
    wv = np.array([[s, s, -s, -s, u, -u / 2, -u / 2, 0.0]], np.float32)

    in_maps = []
    for ci in range(NCORES):
        sl = slice(ROWS * ci, ROWS * (ci + 1))
        xl, yl = x[sl], y[sl]
        xh, xll = _split2(xl)
        yh, yll = _split2(yl)
        lin_x = np.vstack([xh.T.astype(np.float32), xll.T.astype(np.float32),
                           xh.T.astype(np.float32), ones2]).astype(ml_dtypes.bfloat16)
        lin_y = np.vstack([yh.T.astype(np.float32), yll.T.astype(np.float32),
                           yh.T.astype(np.float32), ones2]).astype(ml_dtypes.bfloat16)
        rn_x = (0.5 * np.sum(xl * xl, 1)).astype(np.float32)
        rn_y = (0.5 * np.sum(yl * yl, 1)).astype(np.float32)
        elin_x = np.vstack([(-2 * xl.T).astype(np.float32),
                            np.ones((1, 512), np.float32)]).astype(np.float32)
        elin_y = np.vstack([(-2 * yl.T).astype(np.float32),
                            np.ones((1, 512), np.float32)]).astype(np.float32)
        in_maps.append({
            "lin_x": np.asarray(lin_x), "lin_y": np.asarray(lin_y),
            "rzx": np.asarray(rzx), "rzy": np.asarray(rzy),
            "znx": znx_v.reshape(128, 32), "zny": zny_v.reshape(128, 32),
            "rnx": rn_x.reshape(4, 128).T.copy(),
            "rny": rn_y.reshape(4, 128).T.copy(),
            "elin_x": elin_x, "elin_y": elin_y,
            "erx": erx, "ery": ery,
            "rpow": rpow, "wvec": wv,
        })
    return in_maps


def kernel(pred, target, trace=False):
    x = np.ascontiguousarray(np.asarray(pred, np.float32))
    y = np.ascontiguousarray(np.asarray(target, np.float32))
    assert x.shape == (N, D) and y.shape == (N, D)
    if "nc" not in _CACHE:
        _CACHE["nc"] = _build_nc()
    nc = _CACHE["nc"]
    in_maps = _build_inputs(x, y)
    kwargs = {}
    if trace:
        import tempfile
        kwargs["tmpdir"] = tempfile.mkdtemp(prefix="bass_trace_")
        kernel.last_trace_dir = kwargs["tmpdir"]
    res = run_bass_kernel_spmd(nc, in_maps, core_ids=list(range(NCORES)),
                               trace=trace, **kwargs)
    val = np.float32(res.results[0]["out"][0, 0])
    if trace:
        kernel.last_exec_time_ns = res.exec_time_ns
    return np.asarray(val, np.float32).reshape(())
